# revision 104
# baseline (speedup 1.0000x reference)
r"""DbrxAttention on 8 TRN2 NeuronCores, tensor-parallel across heads.

Per-core shard (core c of 8): 6 query heads (q heads 6c..6c+5), kv head c
(replicated per its 6-head query group), plus the matching 768 input
columns of the out-projection. Each core computes a partial out-proj
(row-parallel Wout); the partials are summed on the host (the all-reduce
of the TP pattern).

All three projections (QKV, out-proj) run as fp8(e4m3) DoubleRow 3-term
compensated matmuls: every operand is split host-side into hi = fp8(x),
lo = fp8(x - hi), and each pair of 128-deep contraction units is computed
as hi*hi + hi*lo + lo*hi DoubleRow instructions (0.5 PE cycles/row,
contraction 256) — 0.75x the fp16 PE time at ~0.15% error. Weights are
pre-scaled (A_SCALE for Wqkv, B_SCALE for Wout) to clear e4m3's denormal
floor; the rope tables, the qkv clip constants, and a final host divide
compensate. Scores stay fp16 (single-unit contraction cannot pair).

Layouts (per core, all device tensors):
  hid8   [2*6144, 2048] fp8  hidden^T hi/lo rows packed per 512-row group
  wq8    [2*6144, 1024] fp8  [q0..q5 | k | v] columns of Wqkv^T, same
  wout8  [2*768,  6144] fp8  Wout[:, shard]^T hi rows then lo rows
  cos/sin tables [128, 2048] fp16, neox rope with sign-folded sin; the
  score scale AND 1/A_SCALE are folded into the single shared table pair
  (k-ropes multiply by sqrt(128) via DVE scalar_tensor_tensor to undo the
  score scale; POOL's scalar_tensor_tensor fails the hw compile).
  masks  [128, 128] fp16  multiplicative causal mask (f >= p) for the
         in-tile triangle of diagonal score tiles

Structure (schedule-sim: 531us, PE ~97% busy):
- Each QKV sweep (512-t chunk) is two passes over the 24 d-pairs reading
  a persistent SBUF fp8 hi/lo slab: pass A computes q0..q5 in three
  2-bank "wide" PSUM tiles; pass B computes k + v in the 2-bank ring.
  During pass B the wides are free, so the previous chunk's attention
  chains interleave into the PE stream (their exp/DVE latency hides
  under k/v GEMM work).
- Attention chains run 2-WAY (two heads share one attnw PSUM tile, one
  half each): the partner head's scores/av act as PE filler inside each
  head's score->exp->av latency chain, which matters because the PE wait
  queue is only 4 deep. Per block: kt-pair scores into one wide PSUM
  tile -> single 1024-wide exp on ACT -> bf16 probs; diagonal kt tiles
  compute only the causal q-suffix with a triangle mask on DVE; row sums
  accumulate on DVE in bf16 legs + one POOL partition_all_reduce per
  chain; each chain yields BEFORE consuming its probs so interleaved
  filler lands between exp and av. Normalization multiplies in place on
  the bf16 au tile, then splits into the fp8 hi/lo attnT planes for the
  DoubleRow out-projection.
- The last chunk's chains run after QKV with the jc<3 out-proj blocks
  rate-interleaved (evacs on DVE to keep ACT free for exps); the jc=3
  tail runs flat-out with ACT/DVE alternating evacs and depth-2 weight
  prefetch. Partial fp16 outputs stream per t-tile; host sums the 8
  partials in fp32 and divides by A_SCALE*B_SCALE.
- Queue routing keeps the in-order DMA queues unblocked: hid slab on
  ACT (first-group startup pairs on SP: ACT pays ~1.3us of
  LoadActFuncSet at t=0), weights on SP (pass B's first tile hoisted
  into pass A), rope partition-swaps + tables on POOL, k-ropes on DVE.
"""

import os

import numpy as np

import concourse.mybir as mybir
import concourse.tile as tile
from concourse import bacc
from concourse import bass_isa
from concourse.bass_utils import run_bass_kernel_spmd

F32R = mybir.dt.float32r
F32 = mybir.dt.float32
F16 = mybir.dt.float16
BF16 = mybir.dt.bfloat16
F8 = mybir.dt.float8e4
DRM = mybir.MatmulPerfMode.DoubleRow

T = 2048
D = 6144
N_HEADS = 48
N_KV = 8
HD = 128
CLIP = 8.0
THETA = 500000.0
N_CORES = 8
HPC = N_HEADS // N_CORES      # q heads per core = 6
QKJ = HPC + 1                 # q+k j-tiles per core = 7
DCH = D // 128                # 48 contraction chunks
DG = DCH // 4                 # 12 batched (4-chunk) DMA groups
TCH = T // 512                # 4 t-chunks
TTILES = T // 128             # 16 t-tiles
OCH = D // 512                # 12 out-proj column chunks
ICH = HPC                     # 6 out-proj contraction chunks (768/128)
A_SCALE = 16.0                # host pre-scale on Wqkv so fp8(e4m3) hi/lo
                              # splits of the 0.02-sigma weights stay out of
                              # the denormal floor; compensated in the rope
                              # tables (q: score_scale/A, k: 1/A), the clip
                              # constants (8*A), and a final host divide for
                              # the v path. Kept at 16 so clipped v (and thus
                              # attn) stays within e4m3's +-240 range when
                              # attnT is stored as fp8 hi/lo.
B_SCALE = 32.0                # same for Wout; the final host sum divides by
                              # A_SCALE * B_SCALE.

_compiled = None


def _build():
    nc = bacc.Bacc("TRN2", target_bir_lowering=False, debug=False,
                   num_devices=N_CORES)

    # hi/lo fp8 pairs packed group-major: rows [g*1024 .. g*1024+511] are
    # the hi rows of d-group g, [+512 .. +1023] the lo rows, so one 3-dim
    # DMA per group fills the slab's [4 hi | 4 lo] plane block.
    hid8_d = nc.dram_tensor("hid8", [2 * D, T], F8, kind="ExternalInput").ap()
    wq8_d = nc.dram_tensor("wq8", [2 * D, 1024], F8, kind="ExternalInput").ap()
    wout8_d = nc.dram_tensor("wout8", [2 * HPC * HD, D], F8,
                             kind="ExternalInput").ap()
    cosk_d = nc.dram_tensor("cosk", [HD, T], F16, kind="ExternalInput").ap()
    sink_d = nc.dram_tensor("sink", [HD, T], F16, kind="ExternalInput").ap()
    mask_d = nc.dram_tensor("maskm", [HD, 128], F16, kind="ExternalInput").ap()
    outp_d = nc.dram_tensor("outp", [T, D], F16, kind="ExternalOutput").ap()

    mn, mx = mybir.AluOpType.min, mybir.AluOpType.max
    mult, add = mybir.AluOpType.mult, mybir.AluOpType.add
    EXP = mybir.ActivationFunctionType.Exp

    with tile.TileContext(nc) as tc:
        with (
            tc.tile_pool(name="sb", bufs=1) as pool,
            tc.tile_pool(name="ps", bufs=1, space="PSUM") as psum,
        ):
            # persistent tensors
            qkT = pool.tile([128, QKJ, T], F16)       # roped q (scaled) + k
            v_sb = pool.tile([128, TTILES, HD], BF16)  # clipped v, [t%128, t//128, hd]
            # normalized attn^T as fp8 hi planes 0..HPC-1, lo planes HPC..
            attnT = pool.tile([128, 2 * HPC, T], F8)
            cosk = pool.tile([HD, T], F16)
            sink = pool.tile([HD, T], F16)
            masks = pool.tile([HD, 128], F16)
            # persistent hid slab: per-sweep writes overwrite slices, so the
            # WAR against the previous sweep's readers is tracked per-slice
            # (a per-sweep pool.tile would bump the whole-tile version and
            # serialize the refill behind all of pass B).
            # fp8 planes in per-group blocks [4 hi | 4 lo]: plane(d) =
            # 8*(d//4) + d%4, lo at +4 — every DoubleRow pair and every
            # group refill DMA is a contiguous slice.
            hslab = pool.tile([128, 2 * DCH, 512], F8)

            def load_tables():
                nc.gpsimd.dma_start(cosk[:], cosk_d[:])
                nc.gpsimd.dma_start(sink[:], sink_d[:])
                nc.gpsimd.dma_start(masks[:], mask_d[:])

            def qkv_sweep(tcx, interleave=None):
                # Pass A: q heads j0..j5 accumulate in three 2-bank "wide"
                # PSUM tiles (2 from the sc2 ring + the attnw tile) over all
                # 48 d-chunks; hid lands in a resident SBUF slab. Pass B:
                # k (j6) and v accumulate in the 2-bank ring re-reading the
                # slab (no second hid DMA). During pass B the wides are free
                # again, so the previous chunk's attention chains interleave
                # into the PE stream (hiding the chain latency that
                # otherwise pays off only after the last sweep).
                # All projection matmuls run as fp8(e4m3) DoubleRow 3-term
                # compensation (hi*hi + hi*lo + lo*hi): each DR instruction
                # contracts a contiguous pair of 128-d units at 0.5
                # cycles/row, so a d-pair costs 3 DRs vs 2 fp16 matmuls =
                # 0.75x PE time at ~0.15% error.
                tsl = slice(tcx * 512, (tcx + 1) * 512)
                widesA = [psum.tile([128, 1024], F32, tag="wide", bufs=2,
                                    name=f"qkw{w}") for w in range(2)]
                widesA.append(psum.tile([128, 1024], F32, tag="attnw", bufs=1,
                                        name="qkw2"))
                qk_ps = [widesA[j // 2][:, (j % 2) * 512:(j % 2 + 1) * 512]
                         for j in range(6)]
                # pass B's first weight tile is DMA'd during pass A so the
                # B-start matmuls never wait on the SP queue
                wqb0 = pool.tile([128, 8, 256], F8, tag="wqb", bufs=3)
                for g in range(DG):
                    g4 = slice(g * 512, (g + 1) * 512)
                    wqa = pool.tile([128, 8, 768], F8, tag="wqa", bufs=2)
                    if tcx == 0 and g == 0:
                        # startup-latency order: the d0/d1 hi pair first (the
                        # first DR reads it), then the lo planes, then d2/d3.
                        # hid DMAs ride the cheap POOL issue queue.
                        # startup path: two-plane DMAs ordered so the
                        # first d-pair's three DR terms (hi*hi, hi*lo,
                        # lo*hi) have operands earliest; SP starts at t=200
                        # while ACT pays ~1.3us of LoadActFuncSet first
                        def pair_rows(half, i2):
                            r0 = half * 512 + 2 * i2 * 128
                            return slice(r0, r0 + 256)

                        def pair_pl(half, i2):
                            return slice(4 * half + 2 * i2,
                                         4 * half + 2 * i2 + 2)

                        def w2(eng, half, i2):
                            eng.dma_start(
                                wqa[:, pair_pl(half, i2), :],
                                wq8_d[pair_rows(half, i2),
                                      0:768].rearrange("(x p) w -> p x w",
                                                       p=128))

                        def h2(eng, half, i2):
                            eng.dma_start(
                                hslab[:, pair_pl(half, i2), :],
                                hid8_d[pair_rows(half, i2),
                                       tsl].rearrange("(x p) t -> p x t",
                                                      p=128))

                        w2(nc.sync, 0, 0)
                        h2(nc.sync, 0, 0)
                        h2(nc.sync, 1, 0)
                        w2(nc.sync, 1, 0)
                        h2(nc.scalar, 0, 1)
                        h2(nc.scalar, 1, 1)
                        w2(nc.sync, 0, 1)
                        w2(nc.sync, 1, 1)
                    else:
                        nc.sync.dma_start(
                            wqa[:, 0:4, :],
                            wq8_d[g * 1024:g * 1024 + 512, 0:768].rearrange(
                                "(x p) w -> p x w", p=128))
                        nc.sync.dma_start(
                            wqa[:, 4:8, :],
                            wq8_d[g * 1024 + 512:(g + 1) * 1024,
                                  0:768].rearrange("(x p) w -> p x w", p=128))
                        nc.scalar.dma_start(
                            hslab[:, 8 * g:8 * g + 4, :],
                            hid8_d[g * 1024:g * 1024 + 512, tsl].rearrange(
                                "(x p) t -> p x t", p=128))
                        nc.scalar.dma_start(
                            hslab[:, 8 * g + 4:8 * g + 8, :],
                            hid8_d[g * 1024 + 512:(g + 1) * 1024,
                                   tsl].rearrange("(x p) t -> p x t", p=128))
                    if g == DG - 1:
                        # issued last so its buffer-ring WAR (prev sweep's
                        # pass-B g10 tile) never head-of-line-blocks the wqa
                        # prefetch stream
                        nc.sync.dma_start(
                            wqb0[:, 0:4, :],
                            wq8_d[0:512, 768:1024].rearrange(
                                "(x p) w -> p x w", p=128))
                        nc.sync.dma_start(
                            wqb0[:, 4:8, :],
                            wq8_d[512:1024, 768:1024].rearrange(
                                "(x p) w -> p x w", p=128))
                    for i2 in range(2):
                        d = g * 4 + 2 * i2
                        wh = slice(2 * i2, 2 * i2 + 2)
                        wl = slice(4 + 2 * i2, 4 + 2 * i2 + 2)
                        hh = slice(8 * g + 2 * i2, 8 * g + 2 * i2 + 2)
                        hl = slice(8 * g + 4 + 2 * i2, 8 * g + 6 + 2 * i2)
                        st, sp = d == 0, d == DCH - 2
                        for j in range(6):
                            jsl = slice(j * 128, (j + 1) * 128)
                            nc.tensor.matmul(qk_ps[j], wqa[:, wh, jsl],
                                             hslab[:, hh, :], start=st,
                                             stop=False, perf_mode=DRM,
                                             skip_group_check=True)
                            nc.tensor.matmul(qk_ps[j], wqa[:, wh, jsl],
                                             hslab[:, hl, :], start=False,
                                             stop=False, perf_mode=DRM,
                                             skip_group_check=True)
                            nc.tensor.matmul(qk_ps[j], wqa[:, wl, jsl],
                                             hslab[:, hh, :], start=False,
                                             stop=sp, perf_mode=DRM,
                                             skip_group_check=True)
                # evac A: clips first (release the wides for the interleaved
                # chains), then ropes for q0..q5 on POOL
                rawsq = []
                for w in range(3):
                    raw2 = pool.tile([128, 1024], F32, tag="raw2", bufs=3,
                                     name=f"raw2_{w}")
                    nc.vector.tensor_scalar(raw2[:], widesA[w][:],
                                            CLIP * A_SCALE, -CLIP * A_SCALE,
                                            mn, mx)
                    rawsq += [raw2[:, 0:512], raw2[:, 512:1024]]

                def rope(j, raw, eng=None):
                    eng = eng or nc.gpsimd
                    xr = pool.tile([128, 512], F32, tag="xr", bufs=2)
                    # SBUF->SBUF partition swap issued from the POOL queue:
                    # keeps the sync queue free so pass B's wqb prefetch
                    # isn't blocked behind 12 swap issues
                    nc.gpsimd.dma_start(xr[0:64, :], raw[64:128, :])
                    nc.gpsimd.dma_start(xr[64:128, :], raw[0:64, :])
                    dst = qkT[:, j, tsl]
                    if j < HPC:
                        eng.tensor_tensor(dst, raw, cosk[:, tsl], mult)
                        eng.tensor_tensor(xr[:], xr[:], sink[:, tsl], mult)
                    else:
                        # tables are q-scaled (score scale folded); k undoes
                        # it via the scalar operand — DVE only, POOL's
                        # scalar_tensor_tensor fails the hw compile
                        s = float(HD ** 0.5)
                        eng.scalar_tensor_tensor(dst, raw, s, cosk[:, tsl],
                                                 mult, mult)
                        eng.scalar_tensor_tensor(xr[:], xr[:], s,
                                                 sink[:, tsl], mult, mult)
                    eng.tensor_tensor(dst, dst, xr[:], add)

                for j in range(6):
                    rope(j, rawsq[j])
                # pass B: k and v from the slab; previous chunk's chains
                # interleave here
                k_ps = psum.tile([128, 512], F32, tag="bank", bufs=2)
                v_ps = psum.tile([128, 512], F32, tag="bank", bufs=2)
                due = 0.0
                n_y = HPC * ((4 * (tcx - 1) + 4) // 2 + 1) if tcx >= 1 else 0
                # drain the interleaved chains ~25% ahead of pass B so the
                # last chain's PSUM (wide/attnw) release never stalls the
                # next sweep's pass A
                rate = 1.0 * n_y / (2 * DG) if interleave is not None else 0.0
                for g in range(DG):
                    g4 = slice(g * 512, (g + 1) * 512)
                    if g == 0:
                        wqb = wqb0
                    else:
                        wqb = pool.tile([128, 8, 256], F8, tag="wqb", bufs=3)
                        nc.sync.dma_start(
                            wqb[:, 0:4, :],
                            wq8_d[g * 1024:g * 1024 + 512,
                                  768:1024].rearrange("(x p) w -> p x w",
                                                      p=128))
                        nc.sync.dma_start(
                            wqb[:, 4:8, :],
                            wq8_d[g * 1024 + 512:(g + 1) * 1024,
                                  768:1024].rearrange("(x p) w -> p x w",
                                                      p=128))
                    for i2 in range(2):
                        d = g * 4 + 2 * i2
                        wh = slice(2 * i2, 2 * i2 + 2)
                        wl = slice(4 + 2 * i2, 4 + 2 * i2 + 2)
                        hh = slice(8 * g + 2 * i2, 8 * g + 2 * i2 + 2)
                        hl = slice(8 * g + 4 + 2 * i2, 8 * g + 6 + 2 * i2)
                        st, sp = d == 0, d == DCH - 2
                        nc.tensor.matmul(k_ps[:], wqb[:, wh, 0:128],
                                         hslab[:, hh, :], start=st,
                                         stop=False, perf_mode=DRM)
                        nc.tensor.matmul(k_ps[:], wqb[:, wh, 0:128],
                                         hslab[:, hl, :], start=False,
                                         stop=False, perf_mode=DRM)
                        nc.tensor.matmul(k_ps[:], wqb[:, wl, 0:128],
                                         hslab[:, hh, :], start=False,
                                         stop=sp, perf_mode=DRM)
                        for s in range(4):
                            # packed quarter-bank outputs: start=True zeroes
                            # the whole 2KB zero-region, so only the first
                            # sub-matmul of the bank may set it
                            s128 = slice(s * 128, (s + 1) * 128)
                            nc.tensor.matmul(v_ps[:, s128],
                                             hslab[:, hh, s128],
                                             wqb[:, wh, 128:256],
                                             start=(st and s == 0),
                                             stop=False, perf_mode=DRM,
                                             skip_group_check=True)
                            nc.tensor.matmul(v_ps[:, s128],
                                             hslab[:, hh, s128],
                                             wqb[:, wl, 128:256],
                                             start=False, stop=False,
                                             perf_mode=DRM,
                                             skip_group_check=True)
                            nc.tensor.matmul(v_ps[:, s128],
                                             hslab[:, hl, s128],
                                             wqb[:, wh, 128:256],
                                             start=False,
                                             stop=(sp and s == 3),
                                             perf_mode=DRM,
                                             skip_group_check=True)
                        due += rate
                        while due >= 1.0:
                            next(interleave, None)
                            due -= 1.0
                if interleave is not None:
                    for _ in interleave:
                        pass
                # evac B
                rawk = pool.tile([128, 512], F32, tag="raw", bufs=2)
                nc.vector.tensor_scalar(rawk[:], k_ps[:], CLIP * A_SCALE,
                                        -CLIP * A_SCALE, mn, mx)
                nc.vector.tensor_scalar(
                    v_sb[:, tcx * 4:(tcx + 1) * 4, :],
                    v_ps[:].rearrange("p (a h) -> p a h", a=4),
                    CLIP * A_SCALE, -CLIP * A_SCALE, mn, mx)
                rope(HPC, rawk[:], eng=nc.vector)

            def attn_chain(h, jc, attn_ps, sfx):
                # generator: yields once per 2-kt block so the driver can
                # interleave ready out-proj matmuls into the in-order PE
                # stream (fills the PE bubble left by the ACT-paced exp).
                # Scores for a kt-pair land in one 2-bank "wide" PSUM tile so
                # a single 1024-wide exp serves both (less ACT overhead).
                # Diagonal-straddle kt tiles (r = kt-4jc >= 0) compute only
                # the causally-needed q-suffix [128r:512] — 15% less
                # score/v PE work; the in-tile triangle is masked by one
                # [128,128] pattern at the suffix head. The unwritten prefix
                # of those PSUM halves holds stale data; exp covers it but
                # nothing downstream reads it.
                qsl = slice(jc * 512, (jc + 1) * 512)
                n_kt = 4 * jc + 4
                n_b = n_kt // 2
                LEAD = 1
                pbs = {}
                # row sums accumulate on DVE in bf16 (2-byte dtype gets the
                # fast DVE mode); suffix-kt adds land in leg 0 (always fully
                # initialized by kt 0), full-width kts alternate legs
                two_legs = jc >= 1
                accs = [pool.tile([128, 512], BF16, tag=f"acc{i}{sfx}",
                                  bufs=1, name=f"acc{i}{sfx}")
                        for i in range(2 if two_legs else 1)]
                accs = accs + accs[:1] if not two_legs else accs
                for bstep in range(n_b + LEAD):
                    if bstep < n_b:
                        b = bstep
                        sc2 = psum.tile([128, 1024], F32, tag="wide", bufs=2)
                        for half in range(2):
                            kt = 2 * b + half
                            r = kt - 4 * jc
                            off = 128 * r if r > 0 else 0
                            nc.tensor.matmul(
                                sc2[:, half * 512 + off:(half + 1) * 512],
                                qkT[:, HPC, kt * 128:(kt + 1) * 128],
                                qkT[:, h, jc * 512 + off:(jc + 1) * 512],
                                start=True, stop=True,
                                skip_group_check=True)
                        pb2 = pool.tile([128, 1024], BF16, tag="pb", bufs=4)
                        nc.scalar.activation(pb2[:], sc2[:], EXP)
                        for half in range(2):
                            kt = 2 * b + half
                            r = kt - 4 * jc
                            if r >= 0:
                                msl = slice(half * 512 + 128 * r,
                                            half * 512 + 128 * r + 128)
                                nc.vector.tensor_tensor(
                                    pb2[:, msl], pb2[:, msl], masks[:], mult)
                        for half in range(2):
                            kt = 2 * b + half
                            r = kt - 4 * jc
                            if r > 0:
                                with nc.allow_low_precision(
                                        reason="bf16 row-sum legs"):
                                    nc.vector.tensor_tensor(
                                        accs[0][:, 128 * r:512],
                                        accs[0][:, 128 * r:512],
                                        pb2[:, half * 512 + 128 * r:
                                            (half + 1) * 512], add)
                            else:
                                leg = accs[kt % 2] if two_legs else accs[0]
                                psl = pb2[:, half * 512:(half + 1) * 512]
                                if kt < 2:
                                    nc.vector.tensor_scalar(
                                        leg[:], psl, 0.0, None, add)
                                else:
                                    with nc.allow_low_precision(
                                            reason="bf16 row-sum legs"):
                                        nc.vector.tensor_tensor(
                                            leg[:], leg[:], psl, add)
                        pbs[b] = pb2
                    # yield BEFORE the av consumption: the driver's filler
                    # matmuls land between exp(b-LEAD) and the av that waits
                    # on it, so the 4-deep PE wait queue never parks on ACT
                    yield
                    if bstep >= LEAD:
                        b = bstep - LEAD
                        pb2 = pbs.pop(b)
                        for half in range(2):
                            kt = 2 * b + half
                            r = kt - 4 * jc
                            off = 128 * r if r > 0 else 0
                            st, sp = kt == 0, kt == n_kt - 1
                            nc.tensor.matmul(
                                attn_ps[:, off:512], v_sb[:, kt, :],
                                pb2[:, half * 512 + off:(half + 1) * 512],
                                start=st, stop=sp, skip_group_check=True)
                if two_legs:
                    with nc.allow_low_precision(
                            reason="bf16 row-sum combine, 2e-2 budget"):
                        nc.vector.tensor_tensor(accs[0][:], accs[0][:],
                                                accs[1][:], add)
                # row sums via POOL partition all-reduce (fp32 internal,
                # broadcast to all partitions for free) — no PE rows spent;
                # normalize off the critical path, all-bf16 for fast DVE
                au = pool.tile([128, 512], BF16, tag="au", bufs=4)
                nc.scalar.copy(au[:], attn_ps[:])
                allsum = pool.tile([128, 512], BF16, tag="rec", bufs=4)
                nc.gpsimd.partition_all_reduce(allsum[:], accs[0][:], 128,
                                               bass_isa.ReduceOp.add)
                recb = pool.tile([128, 512], BF16, tag="recb", bufs=4)
                with nc.allow_low_precision(
                        reason="bf16 softmax scale, 2e-2 budget"):
                    nc.vector.reciprocal(recb[:], allsum[:])
                # normalized attn lands as an fp8 hi/lo pair for the
                # DoubleRow out-projection: hi = fp8(a), lo = fp8(a - hi).
                # The multiply runs in place on au (bf16) to save SBUF.
                with nc.allow_low_precision(
                        reason="fp8 hi/lo split, compensated"):
                    nc.vector.tensor_tensor(au[:], au[:], recb[:], mult)
                    nc.scalar.copy(attnT[:, h, qsl], au[:])
                    nc.vector.tensor_tensor(attnT[:, HPC + h, qsl], au[:],
                                            attnT[:, h, qsl],
                                            mybir.AluOpType.subtract)

            def outproj_blocks(pairs, depth=2, evac_dve=False):
                # flat generator of out-proj (oc, tt) blocks over the given
                # (jc, oc) pairs; drained one block per chain step so PE
                # never idles while exp paces the chains. wo weight tiles
                # prefetch `depth` oc's ahead (bufs=4 serves both live
                # generators), so no block waits on its weight transfer.
                # PSUM->SBUF evacs go to DVE on the chain-interleaved
                # portion (ACT stays free for the exps the avs wait on) and
                # alternate ACT/DVE on the tail (GPSIMD cannot read PSUM).
                # oc-major across the t-groups: one wo load serves every
                # group's blocks for that column chunk (3x less weight DMA
                # on the interleaved portion)
                def load_wo(oc):
                    wo = pool.tile([128, 2 * ICH, 512], F8, tag="wo", bufs=4)
                    osl = slice(oc * 512, (oc + 1) * 512)
                    nc.sync.dma_start(
                        wo[:, 0:ICH, :],
                        wout8_d[0:768, osl].rearrange("(x p) o -> p x o",
                                                      p=128))
                    nc.sync.dma_start(
                        wo[:, ICH:2 * ICH, :],
                        wout8_d[768:1536, osl].rearrange("(x p) o -> p x o",
                                                         p=128))
                    return wo

                # distinct oc sequence for prefetch (depth 2, bufs=3)
                oc_seq = []
                for _, oc in pairs:
                    if not oc_seq or oc_seq[-1] != oc:
                        oc_seq.append(oc)
                pending = [load_wo(oc) for oc in oc_seq[:depth]]
                nfetched = [min(depth, len(oc_seq))]

                def block(out_ps, wo, t):
                    tsl8 = slice(t * 128, (t + 1) * 128)
                    for i2 in range(ICH // 2):
                        i = 2 * i2
                        ah = attnT[:, i:i + 2, tsl8]
                        al = attnT[:, HPC + i:HPC + i + 2, tsl8]
                        wh_ = wo[:, i:i + 2, :]
                        wl_ = wo[:, ICH + i:ICH + i + 2, :]
                        nc.tensor.matmul(out_ps[:], ah, wh_,
                                         start=(i2 == 0), stop=False,
                                         perf_mode=DRM)
                        nc.tensor.matmul(out_ps[:], ah, wl_,
                                         start=False, stop=False,
                                         perf_mode=DRM)
                        nc.tensor.matmul(out_ps[:], al, wh_,
                                         start=False,
                                         stop=(i2 == ICH // 2 - 1),
                                         perf_mode=DRM)

                def gen():
                    wo = None
                    last_oc = None
                    for n, (jc, oc) in enumerate(pairs):
                        if oc != last_oc:
                            if nfetched[0] < len(oc_seq):
                                pending.append(load_wo(oc_seq[nfetched[0]]))
                                nfetched[0] += 1
                            wo = pending.pop(0)
                            last_oc = oc
                        osl = slice(oc * 512, (oc + 1) * 512)
                        for tt in range(4):
                            t = 4 * jc + tt
                            out_ps = psum.tile([128, 512], F32,
                                               tag="bank", bufs=2)
                            block(out_ps, wo, t)
                            osb = pool.tile([128, 512], F16, tag="osb",
                                            bufs=4)
                            th = slice(jc * 512 + tt * 128,
                                       jc * 512 + (tt + 1) * 128)
                            if (evac_dve and jc < TCH - 1) or \
                                    (oc + tt) % 2 != 0:
                                nc.vector.tensor_copy(osb[:], out_ps[:])
                            else:
                                nc.scalar.copy(osb[:], out_ps[:])
                            nc.sync.dma_start(outp_d[th, osl], osb[:])
                            yield

                return gen()

            # ---- Sweeps with the previous chunk's chains interleaved
            # into pass B; post-QKV: last chunk's chains with all out-proj
            # groups interleaved into the PE stream. Chains run 2-way (one
            # head per attnw half): each head's exp gets the other head's
            # scores/av as extra PE filler, halving exp-latency stalls ----
            def chain_group(jc):
                for h0 in range(0, HPC, 2):
                    attnw = psum.tile([128, 1024], F32, tag="attnw", bufs=1)
                    gens = [attn_chain(h0, jc, attnw[:, 0:512], "a"),
                            attn_chain(h0 + 1, jc, attnw[:, 512:1024], "b")]
                    alive = [True, True]
                    while alive[0] or alive[1]:
                        for i, g in enumerate(gens):
                            if alive[i]:
                                try:
                                    next(g)
                                    yield
                                except StopIteration:
                                    alive[i] = False

            load_tables()
            qkv_sweep(0)
            for tcx in range(1, TCH):
                qkv_sweep(tcx, interleave=chain_group(tcx - 1))
            op_pairs = [(jc, oc) for oc in range(OCH)
                        for jc in range(TCH - 1)]
            ops = outproj_blocks(op_pairs, evac_dve=True)
            due = 0.0
            rate = 0.7 * (3 * 4 * OCH) / (HPC * ((4 * 3 + 4) // 2 + 1))
            for _ in chain_group(TCH - 1):
                due += rate
                while due >= 1.0:
                    next(ops, None)
                    due -= 1.0
            # create the last group's generator before draining the rest so
            # its first weight tile is already in flight
            tail = outproj_blocks([(TCH - 1, oc) for oc in range(OCH)],
                                  depth=2)
            for _ in ops:
                pass
            for _ in tail:
                pass

    nc.compile()
    return nc


def kernel(hidden_states, position_ids, Wqkv, Wout):
    global _compiled
    hidden_states = np.asarray(hidden_states, dtype=np.float32)
    position_ids = np.asarray(position_ids).astype(np.int64)
    Wqkv = np.asarray(Wqkv, dtype=np.float32)
    Wout = np.asarray(Wout, dtype=np.float32)

    if _compiled is None:
        _compiled = _build()
    nc = _compiled

    import ml_dtypes
    E4M3 = ml_dtypes.float8_e4m3

    def split8(x):
        hi = x.astype(E4M3)
        lo = (x - hi.astype(np.float32)).astype(E4M3)
        return np.ascontiguousarray(hi), np.ascontiguousarray(lo)

    # host prep: rope tables (from actual position_ids), masks, shards.
    # Wqkv is pre-scaled by A_SCALE for the fp8 split; the q tables fold
    # score_scale/A_SCALE, the k tables 1/A_SCALE, and the v path's factor
    # is divided out of the final host sum.
    scale = HD ** -0.5
    half = HD // 2
    inv_freq = 1.0 / (THETA ** (np.arange(half, dtype=np.float64) / half))
    freqs = position_ids.astype(np.float64)[None, :] * inv_freq[:, None]  # [64, T]
    cos = np.cos(freqs)
    sin = np.sin(freqs)
    cosf = np.concatenate([cos, cos], 0)
    sinf = np.concatenate([-sin, sin], 0)
    cosk = (cosf * (scale / A_SCALE)).astype(np.float16)
    sink = (sinf * (scale / A_SCALE)).astype(np.float16)

    p = np.arange(128)[:, None]
    f = np.arange(128)[None, :]
    masks = (f >= p).astype(np.float16)

    def pack_groups(hi, lo):
        # per-512-row group: [hi rows | lo rows] -> [2*rows, cols]
        ng = hi.shape[0] // 512
        out = np.stack([hi.reshape(ng, 512, -1), lo.reshape(ng, 512, -1)],
                       axis=1)
        return np.ascontiguousarray(out.reshape(2 * hi.shape[0], hi.shape[1]))

    hidT = np.ascontiguousarray(hidden_states.T)
    hid8 = pack_groups(*split8(hidT))

    q_size = N_HEADS * HD
    in_maps = []
    for c in range(N_CORES):
        qrows = Wqkv[c * HPC * HD:(c + 1) * HPC * HD]
        krows = Wqkv[q_size + c * HD:q_size + (c + 1) * HD]
        vrows = Wqkv[q_size + N_KV * HD + c * HD:q_size + N_KV * HD + (c + 1) * HD]
        wqkvT = np.ascontiguousarray(
            np.concatenate([qrows, krows, vrows], 0).T) * A_SCALE
        wq8 = pack_groups(*split8(wqkvT))
        woutT = np.ascontiguousarray(
            Wout[:, c * HPC * HD:(c + 1) * HPC * HD].T) * B_SCALE
        wout8 = np.ascontiguousarray(np.concatenate(split8(woutT), axis=0))
        in_maps.append({
            "hid8": hid8, "wq8": wq8, "wout8": wout8,
            "cosk": cosk, "sink": sink,
            "maskm": masks,
        })

    trace = os.environ.get("DBRX_TRACE", "0") == "1"
    res = run_bass_kernel_spmd(nc, in_maps, core_ids=list(range(N_CORES)),
                               trace=trace)
    kernel.last_result = res

    out = res.results[0]["outp"].astype(np.float32)
    for c in range(1, N_CORES):
        out += res.results[c]["outp"].astype(np.float32)
    # undo the v-path A_SCALE carried through attnT and the Wout B_SCALE
    out /= A_SCALE * B_SCALE
    return out



# revision 110
# speedup vs baseline: 1.0043x; 1.0043x over previous
r"""DbrxAttention on 8 TRN2 NeuronCores, tensor-parallel across heads.

Per-core shard (core c of 8): 6 query heads (q heads 6c..6c+5), kv head c
(replicated per its 6-head query group), plus the matching 768 input
columns of the out-projection. Each core computes a partial out-proj
(row-parallel Wout); the partials are summed on the host (the all-reduce
of the TP pattern).

All three projections (QKV, out-proj) run as fp8(e4m3) DoubleRow 3-term
compensated matmuls: every operand is split host-side into hi = fp8(x),
lo = fp8(x - hi), and each pair of 128-deep contraction units is computed
as hi*hi + hi*lo + lo*hi DoubleRow instructions (0.5 PE cycles/row,
contraction 256) — 0.75x the fp16 PE time at ~0.15% error. Weights are
pre-scaled (A_SCALE for Wqkv, B_SCALE for Wout) to clear e4m3's denormal
floor; the rope tables, the qkv clip constants, and a final host divide
compensate. Scores stay fp16 (single-unit contraction cannot pair).

Layouts (per core, all device tensors):
  hid8   [2*6144, 2048] fp8  hidden^T hi/lo rows packed per 512-row group
  wq8    [2*6144, 1024] fp8  [q0..q5 | k | v] columns of Wqkv^T, same
  wout8  [2*768,  6144] fp8  Wout[:, shard]^T hi rows then lo rows
  cos/sin tables [128, 2048] fp16, neox rope with sign-folded sin; the
  score scale AND 1/A_SCALE are folded into the single shared table pair
  (k-ropes multiply by sqrt(128) via DVE scalar_tensor_tensor to undo the
  score scale; POOL's scalar_tensor_tensor fails the hw compile).
  masks  [128, 128] fp16  multiplicative causal mask (f >= p) for the
         in-tile triangle of diagonal score tiles

Structure (schedule-sim: 531us, PE ~97% busy):
- Each QKV sweep (512-t chunk) is two passes over the 24 d-pairs reading
  a persistent SBUF fp8 hi/lo slab: pass A computes q0..q5 in three
  2-bank "wide" PSUM tiles; pass B computes k + v in the 2-bank ring.
  During pass B the wides are free, so the previous chunk's attention
  chains interleave into the PE stream (their exp/DVE latency hides
  under k/v GEMM work).
- Attention chains run 2-WAY (two heads share one attnw PSUM tile, one
  half each): the partner head's scores/av act as PE filler inside each
  head's score->exp->av latency chain, which matters because the PE wait
  queue is only 4 deep. Per block: kt-pair scores into one wide PSUM
  tile -> single 1024-wide exp on ACT -> bf16 probs; diagonal kt tiles
  compute only the causal q-suffix with a triangle mask on DVE; row sums
  accumulate on DVE in bf16 legs + one POOL partition_all_reduce per
  chain; each chain yields BEFORE consuming its probs so interleaved
  filler lands between exp and av. Normalization multiplies in place on
  the bf16 au tile, then splits into the fp8 hi/lo attnT planes for the
  DoubleRow out-projection.
- The last chunk's chains run after QKV with the jc<3 out-proj blocks
  rate-interleaved (evacs on DVE to keep ACT free for exps); the jc=3
  tail runs flat-out with ACT/DVE alternating evacs and depth-2 weight
  prefetch. Partial fp16 outputs stream per t-tile; host sums the 8
  partials in fp32 and divides by A_SCALE*B_SCALE.
- Queue routing keeps the in-order DMA queues unblocked: hid slab on
  ACT (first-group startup pairs on SP: ACT pays ~1.3us of
  LoadActFuncSet at t=0), weights on SP (pass B's first tile hoisted
  into pass A), rope partition-swaps + tables on POOL, k-ropes on DVE.
"""

import os

import numpy as np

import concourse.mybir as mybir
import concourse.tile as tile
from concourse import bacc
from concourse import bass_isa
from concourse.bass_utils import run_bass_kernel_spmd

F32R = mybir.dt.float32r
F32 = mybir.dt.float32
F16 = mybir.dt.float16
BF16 = mybir.dt.bfloat16
F8 = mybir.dt.float8e4
DRM = mybir.MatmulPerfMode.DoubleRow

T = 2048
D = 6144
N_HEADS = 48
N_KV = 8
HD = 128
CLIP = 8.0
THETA = 500000.0
N_CORES = 8
HPC = N_HEADS // N_CORES      # q heads per core = 6
QKJ = HPC + 1                 # q+k j-tiles per core = 7
DCH = D // 128                # 48 contraction chunks
DG = DCH // 4                 # 12 batched (4-chunk) DMA groups
TCH = T // 512                # 4 t-chunks
TTILES = T // 128             # 16 t-tiles
OCH = D // 512                # 12 out-proj column chunks
ICH = HPC                     # 6 out-proj contraction chunks (768/128)
A_SCALE = 16.0                # host pre-scale on Wqkv so fp8(e4m3) hi/lo
                              # splits of the 0.02-sigma weights stay out of
                              # the denormal floor; compensated in the rope
                              # tables (q: score_scale/A, k: 1/A), the clip
                              # constants (8*A), and a final host divide for
                              # the v path. Kept at 16 so clipped v (and thus
                              # attn) stays within e4m3's +-240 range when
                              # attnT is stored as fp8 hi/lo.
B_SCALE = 32.0                # same for Wout; the final host sum divides by
                              # A_SCALE * B_SCALE.

_compiled = None


def _build():
    nc = bacc.Bacc("TRN2", target_bir_lowering=False, debug=False,
                   num_devices=N_CORES)

    # hi/lo fp8 pairs packed group-major: rows [g*1024 .. g*1024+511] are
    # the hi rows of d-group g, [+512 .. +1023] the lo rows, so one 3-dim
    # DMA per group fills the slab's [4 hi | 4 lo] plane block.
    hid8_d = nc.dram_tensor("hid8", [2 * D, T], F8, kind="ExternalInput").ap()
    wq8_d = nc.dram_tensor("wq8", [2 * D, 1024], F8, kind="ExternalInput").ap()
    wout8_d = nc.dram_tensor("wout8", [2 * HPC * HD, D], F8,
                             kind="ExternalInput").ap()
    cosk_d = nc.dram_tensor("cosk", [HD, T], F16, kind="ExternalInput").ap()
    sink_d = nc.dram_tensor("sink", [HD, T], F16, kind="ExternalInput").ap()
    mask_d = nc.dram_tensor("maskm", [HD, 128], F16, kind="ExternalInput").ap()
    outp_d = nc.dram_tensor("outp", [T, D], F16, kind="ExternalOutput").ap()

    mn, mx = mybir.AluOpType.min, mybir.AluOpType.max
    mult, add = mybir.AluOpType.mult, mybir.AluOpType.add
    EXP = mybir.ActivationFunctionType.Exp

    with tile.TileContext(nc) as tc:
        with (
            tc.tile_pool(name="sb", bufs=1) as pool,
            tc.tile_pool(name="ps", bufs=1, space="PSUM") as psum,
        ):
            # persistent tensors
            qkT = pool.tile([128, QKJ, T], F16)       # roped q (scaled) + k
            v_sb = pool.tile([128, TTILES, HD], BF16)  # clipped v, [t%128, t//128, hd]
            # normalized attn^T as fp8 hi planes 0..HPC-1, lo planes HPC..
            attnT = pool.tile([128, 2 * HPC, T], F8)
            cosk = pool.tile([HD, T], F16)
            sink = pool.tile([HD, T], F16)
            masks = pool.tile([HD, 128], F16)
            # persistent hid slab: per-sweep writes overwrite slices, so the
            # WAR against the previous sweep's readers is tracked per-slice
            # (a per-sweep pool.tile would bump the whole-tile version and
            # serialize the refill behind all of pass B).
            # fp8 planes in per-group blocks [4 hi | 4 lo]: plane(d) =
            # 8*(d//4) + d%4, lo at +4 — every DoubleRow pair and every
            # group refill DMA is a contiguous slice.
            hslab = pool.tile([128, 2 * DCH, 512], F8)

            def load_tables():
                nc.gpsimd.dma_start(cosk[:], cosk_d[:])
                nc.gpsimd.dma_start(sink[:], sink_d[:])
                nc.gpsimd.dma_start(masks[:], mask_d[:])

            def qkv_sweep(tcx, interleave=None):
                # Pass A: q heads j0..j5 accumulate in three 2-bank "wide"
                # PSUM tiles (2 from the sc2 ring + the attnw tile) over all
                # 48 d-chunks; hid lands in a resident SBUF slab. Pass B:
                # k (j6) and v accumulate in the 2-bank ring re-reading the
                # slab (no second hid DMA). During pass B the wides are free
                # again, so the previous chunk's attention chains interleave
                # into the PE stream (hiding the chain latency that
                # otherwise pays off only after the last sweep).
                # All projection matmuls run as fp8(e4m3) DoubleRow 3-term
                # compensation (hi*hi + hi*lo + lo*hi): each DR instruction
                # contracts a contiguous pair of 128-d units at 0.5
                # cycles/row, so a d-pair costs 3 DRs vs 2 fp16 matmuls =
                # 0.75x PE time at ~0.15% error.
                tsl = slice(tcx * 512, (tcx + 1) * 512)
                widesA = [psum.tile([128, 1024], F32, tag="wide", bufs=2,
                                    name=f"qkw{w}") for w in range(2)]
                widesA.append(psum.tile([128, 1024], F32, tag="attnw", bufs=1,
                                        name="qkw2"))
                qk_ps = [widesA[j // 2][:, (j % 2) * 512:(j % 2 + 1) * 512]
                         for j in range(6)]
                # pass B's first weight tile is DMA'd during pass A so the
                # B-start matmuls never wait on the SP queue
                wqb0 = pool.tile([128, 8, 256], F8, tag="wqb", bufs=3)
                for g in range(DG):
                    g4 = slice(g * 512, (g + 1) * 512)
                    wqa = pool.tile([128, 8, 768], F8, tag="wqa", bufs=2)
                    if tcx == 0 and g == 0:
                        # startup-latency order: the d0/d1 hi pair first (the
                        # first DR reads it), then the lo planes, then d2/d3.
                        # hid DMAs ride the cheap POOL issue queue.
                        # startup path: two-plane DMAs ordered so the
                        # first d-pair's three DR terms (hi*hi, hi*lo,
                        # lo*hi) have operands earliest; SP starts at t=200
                        # while ACT pays ~1.3us of LoadActFuncSet first
                        def pair_rows(half, i2):
                            r0 = half * 512 + 2 * i2 * 128
                            return slice(r0, r0 + 256)

                        def pair_pl(half, i2):
                            return slice(4 * half + 2 * i2,
                                         4 * half + 2 * i2 + 2)

                        def w2(eng, half, i2):
                            eng.dma_start(
                                wqa[:, pair_pl(half, i2), :],
                                wq8_d[pair_rows(half, i2),
                                      0:768].rearrange("(x p) w -> p x w",
                                                       p=128))

                        def h2(eng, half, i2):
                            eng.dma_start(
                                hslab[:, pair_pl(half, i2), :],
                                hid8_d[pair_rows(half, i2),
                                       tsl].rearrange("(x p) t -> p x t",
                                                      p=128))

                        w2(nc.sync, 0, 0)
                        h2(nc.sync, 0, 0)
                        h2(nc.sync, 1, 0)
                        w2(nc.sync, 1, 0)
                        h2(nc.scalar, 0, 1)
                        h2(nc.scalar, 1, 1)
                        w2(nc.sync, 0, 1)
                        w2(nc.sync, 1, 1)
                    else:
                        nc.sync.dma_start(
                            wqa[:, 0:4, :],
                            wq8_d[g * 1024:g * 1024 + 512, 0:768].rearrange(
                                "(x p) w -> p x w", p=128))
                        nc.sync.dma_start(
                            wqa[:, 4:8, :],
                            wq8_d[g * 1024 + 512:(g + 1) * 1024,
                                  0:768].rearrange("(x p) w -> p x w", p=128))
                        nc.scalar.dma_start(
                            hslab[:, 8 * g:8 * g + 4, :],
                            hid8_d[g * 1024:g * 1024 + 512, tsl].rearrange(
                                "(x p) t -> p x t", p=128))
                        nc.scalar.dma_start(
                            hslab[:, 8 * g + 4:8 * g + 8, :],
                            hid8_d[g * 1024 + 512:(g + 1) * 1024,
                                   tsl].rearrange("(x p) t -> p x t", p=128))
                    if g == DG - 1:
                        # issued last so its buffer-ring WAR (prev sweep's
                        # pass-B g10 tile) never head-of-line-blocks the wqa
                        # prefetch stream
                        nc.sync.dma_start(
                            wqb0[:, 0:4, :],
                            wq8_d[0:512, 768:1024].rearrange(
                                "(x p) w -> p x w", p=128))
                        nc.sync.dma_start(
                            wqb0[:, 4:8, :],
                            wq8_d[512:1024, 768:1024].rearrange(
                                "(x p) w -> p x w", p=128))
                    for i2 in range(2):
                        d = g * 4 + 2 * i2
                        wh = slice(2 * i2, 2 * i2 + 2)
                        wl = slice(4 + 2 * i2, 4 + 2 * i2 + 2)
                        hh = slice(8 * g + 2 * i2, 8 * g + 2 * i2 + 2)
                        hl = slice(8 * g + 4 + 2 * i2, 8 * g + 6 + 2 * i2)
                        st, sp = d == 0, d == DCH - 2
                        for j in range(6):
                            jsl = slice(j * 128, (j + 1) * 128)
                            nc.tensor.matmul(qk_ps[j], wqa[:, wh, jsl],
                                             hslab[:, hh, :], start=st,
                                             stop=False, perf_mode=DRM,
                                             skip_group_check=True)
                            nc.tensor.matmul(qk_ps[j], wqa[:, wh, jsl],
                                             hslab[:, hl, :], start=False,
                                             stop=False, perf_mode=DRM,
                                             skip_group_check=True)
                            nc.tensor.matmul(qk_ps[j], wqa[:, wl, jsl],
                                             hslab[:, hh, :], start=False,
                                             stop=sp, perf_mode=DRM,
                                             skip_group_check=True)
                # evac A: clips first (release the wides for the interleaved
                # chains), then ropes for q0..q5 on POOL
                rawsq = []
                for w in range(3):
                    raw2 = pool.tile([128, 1024], F32, tag="raw2", bufs=3,
                                     name=f"raw2_{w}")
                    nc.vector.tensor_scalar(raw2[:], widesA[w][:],
                                            CLIP * A_SCALE, -CLIP * A_SCALE,
                                            mn, mx)
                    rawsq += [raw2[:, 0:512], raw2[:, 512:1024]]

                def rope(j, raw, eng=None):
                    eng = eng or nc.gpsimd
                    xr = pool.tile([128, 512], F32, tag="xr", bufs=2)
                    # SBUF->SBUF partition swap issued from the POOL queue:
                    # keeps the sync queue free so pass B's wqb prefetch
                    # isn't blocked behind 12 swap issues
                    nc.gpsimd.dma_start(xr[0:64, :], raw[64:128, :])
                    nc.gpsimd.dma_start(xr[64:128, :], raw[0:64, :])
                    dst = qkT[:, j, tsl]
                    if j < HPC:
                        eng.tensor_tensor(dst, raw, cosk[:, tsl], mult)
                        eng.tensor_tensor(xr[:], xr[:], sink[:, tsl], mult)
                    else:
                        # tables are q-scaled (score scale folded); k undoes
                        # it via the scalar operand — DVE only, POOL's
                        # scalar_tensor_tensor fails the hw compile
                        s = float(HD ** 0.5)
                        eng.scalar_tensor_tensor(dst, raw, s, cosk[:, tsl],
                                                 mult, mult)
                        eng.scalar_tensor_tensor(xr[:], xr[:], s,
                                                 sink[:, tsl], mult, mult)
                    eng.tensor_tensor(dst, dst, xr[:], add)

                for j in range(6):
                    rope(j, rawsq[j])
                # pass B: k and v from the slab; previous chunk's chains
                # interleave here
                k_ps = psum.tile([128, 512], F32, tag="bank", bufs=2)
                v_ps = psum.tile([128, 512], F32, tag="bank", bufs=2)
                due = 0.0
                n_y = HPC * ((4 * (tcx - 1) + 4) // 2 + 1) if tcx >= 1 else 0
                # drain the interleaved chains ~25% ahead of pass B so the
                # last chain's PSUM (wide/attnw) release never stalls the
                # next sweep's pass A
                rate = 1.0 * n_y / (2 * DG) if interleave is not None else 0.0
                for g in range(DG):
                    g4 = slice(g * 512, (g + 1) * 512)
                    if g == 0:
                        wqb = wqb0
                    else:
                        wqb = pool.tile([128, 8, 256], F8, tag="wqb", bufs=3)
                        nc.sync.dma_start(
                            wqb[:, 0:4, :],
                            wq8_d[g * 1024:g * 1024 + 512,
                                  768:1024].rearrange("(x p) w -> p x w",
                                                      p=128))
                        nc.sync.dma_start(
                            wqb[:, 4:8, :],
                            wq8_d[g * 1024 + 512:(g + 1) * 1024,
                                  768:1024].rearrange("(x p) w -> p x w",
                                                      p=128))
                    for i2 in range(2):
                        d = g * 4 + 2 * i2
                        wh = slice(2 * i2, 2 * i2 + 2)
                        wl = slice(4 + 2 * i2, 4 + 2 * i2 + 2)
                        hh = slice(8 * g + 2 * i2, 8 * g + 2 * i2 + 2)
                        hl = slice(8 * g + 4 + 2 * i2, 8 * g + 6 + 2 * i2)
                        st, sp = d == 0, d == DCH - 2
                        nc.tensor.matmul(k_ps[:], wqb[:, wh, 0:128],
                                         hslab[:, hh, :], start=st,
                                         stop=False, perf_mode=DRM)
                        nc.tensor.matmul(k_ps[:], wqb[:, wh, 0:128],
                                         hslab[:, hl, :], start=False,
                                         stop=False, perf_mode=DRM)
                        nc.tensor.matmul(k_ps[:], wqb[:, wl, 0:128],
                                         hslab[:, hh, :], start=False,
                                         stop=sp, perf_mode=DRM)
                        for s in range(4):
                            # packed quarter-bank outputs: start=True zeroes
                            # the whole 2KB zero-region, so only the first
                            # sub-matmul of the bank may set it
                            s128 = slice(s * 128, (s + 1) * 128)
                            nc.tensor.matmul(v_ps[:, s128],
                                             hslab[:, hh, s128],
                                             wqb[:, wh, 128:256],
                                             start=(st and s == 0),
                                             stop=False, perf_mode=DRM,
                                             skip_group_check=True)
                            nc.tensor.matmul(v_ps[:, s128],
                                             hslab[:, hh, s128],
                                             wqb[:, wl, 128:256],
                                             start=False, stop=False,
                                             perf_mode=DRM,
                                             skip_group_check=True)
                            nc.tensor.matmul(v_ps[:, s128],
                                             hslab[:, hl, s128],
                                             wqb[:, wh, 128:256],
                                             start=False,
                                             stop=(sp and s == 3),
                                             perf_mode=DRM,
                                             skip_group_check=True)
                        due += rate
                        while due >= 1.0:
                            next(interleave, None)
                            due -= 1.0
                if interleave is not None:
                    for _ in interleave:
                        pass
                # evac B
                rawk = pool.tile([128, 512], F32, tag="raw", bufs=2)
                nc.vector.tensor_scalar(rawk[:], k_ps[:], CLIP * A_SCALE,
                                        -CLIP * A_SCALE, mn, mx)
                nc.vector.tensor_scalar(
                    v_sb[:, tcx * 4:(tcx + 1) * 4, :],
                    v_ps[:].rearrange("p (a h) -> p a h", a=4),
                    CLIP * A_SCALE, -CLIP * A_SCALE, mn, mx)
                rope(HPC, rawk[:], eng=nc.vector)

            def attn_chain(h, jc, attn_ps, sfx):
                # generator: yields once per 2-kt block so the driver can
                # interleave ready out-proj matmuls into the in-order PE
                # stream (fills the PE bubble left by the ACT-paced exp).
                # Scores for a kt-pair land in one 2-bank "wide" PSUM tile so
                # a single 1024-wide exp serves both (less ACT overhead).
                # Diagonal-straddle kt tiles (r = kt-4jc >= 0) compute only
                # the causally-needed q-suffix [128r:512] — 15% less
                # score/v PE work; the in-tile triangle is masked by one
                # [128,128] pattern at the suffix head. The unwritten prefix
                # of those PSUM halves holds stale data; exp covers it but
                # nothing downstream reads it.
                qsl = slice(jc * 512, (jc + 1) * 512)
                n_kt = 4 * jc + 4
                n_b = n_kt // 2
                LEAD = 1
                pbs = {}
                # row sums accumulate on DVE in bf16 (2-byte dtype gets the
                # fast DVE mode); suffix-kt adds land in leg 0 (always fully
                # initialized by kt 0), full-width kts alternate legs
                two_legs = jc >= 1
                accs = [pool.tile([128, 512], BF16, tag=f"acc{i}{sfx}",
                                  bufs=1, name=f"acc{i}{sfx}")
                        for i in range(2 if two_legs else 1)]
                accs = accs + accs[:1] if not two_legs else accs
                for bstep in range(n_b + LEAD):
                    if bstep < n_b:
                        b = bstep
                        sc2 = psum.tile([128, 1024], F32, tag="wide", bufs=2)
                        for half in range(2):
                            kt = 2 * b + half
                            r = kt - 4 * jc
                            off = 128 * r if r > 0 else 0
                            nc.tensor.matmul(
                                sc2[:, half * 512 + off:(half + 1) * 512],
                                qkT[:, HPC, kt * 128:(kt + 1) * 128],
                                qkT[:, h, jc * 512 + off:(jc + 1) * 512],
                                start=True, stop=True,
                                skip_group_check=True)
                        pb2 = pool.tile([128, 1024], BF16, tag="pb", bufs=4)
                        nc.scalar.activation(pb2[:], sc2[:], EXP)
                        for half in range(2):
                            kt = 2 * b + half
                            r = kt - 4 * jc
                            if r >= 0:
                                msl = slice(half * 512 + 128 * r,
                                            half * 512 + 128 * r + 128)
                                nc.vector.tensor_tensor(
                                    pb2[:, msl], pb2[:, msl], masks[:], mult)
                        for half in range(2):
                            kt = 2 * b + half
                            r = kt - 4 * jc
                            if r > 0:
                                with nc.allow_low_precision(
                                        reason="bf16 row-sum legs"):
                                    nc.vector.tensor_tensor(
                                        accs[0][:, 128 * r:512],
                                        accs[0][:, 128 * r:512],
                                        pb2[:, half * 512 + 128 * r:
                                            (half + 1) * 512], add)
                            else:
                                leg = accs[kt % 2] if two_legs else accs[0]
                                psl = pb2[:, half * 512:(half + 1) * 512]
                                if kt < 2:
                                    nc.vector.tensor_scalar(
                                        leg[:], psl, 0.0, None, add)
                                else:
                                    with nc.allow_low_precision(
                                            reason="bf16 row-sum legs"):
                                        nc.vector.tensor_tensor(
                                            leg[:], leg[:], psl, add)
                        pbs[b] = pb2
                    # yield BEFORE the av consumption: the driver's filler
                    # matmuls land between exp(b-LEAD) and the av that waits
                    # on it, so the 4-deep PE wait queue never parks on ACT
                    yield
                    if bstep >= LEAD:
                        b = bstep - LEAD
                        pb2 = pbs.pop(b)
                        for half in range(2):
                            kt = 2 * b + half
                            r = kt - 4 * jc
                            off = 128 * r if r > 0 else 0
                            st, sp = kt == 0, kt == n_kt - 1
                            nc.tensor.matmul(
                                attn_ps[:, off:512], v_sb[:, kt, :],
                                pb2[:, half * 512 + off:(half + 1) * 512],
                                start=st, stop=sp, skip_group_check=True)
                if two_legs:
                    with nc.allow_low_precision(
                            reason="bf16 row-sum combine, 2e-2 budget"):
                        nc.vector.tensor_tensor(accs[0][:], accs[0][:],
                                                accs[1][:], add)
                # row sums via POOL partition all-reduce (fp32 internal,
                # broadcast to all partitions for free) — no PE rows spent;
                # normalize off the critical path, all-bf16 for fast DVE
                au = pool.tile([128, 512], BF16, tag="au", bufs=4)
                nc.scalar.copy(au[:], attn_ps[:])
                allsum = pool.tile([128, 512], BF16, tag="rec", bufs=4)
                nc.gpsimd.partition_all_reduce(allsum[:], accs[0][:], 128,
                                               bass_isa.ReduceOp.add)
                recb = pool.tile([128, 512], BF16, tag="recb", bufs=4)
                with nc.allow_low_precision(
                        reason="bf16 softmax scale, 2e-2 budget"):
                    nc.vector.reciprocal(recb[:], allsum[:])
                # normalized attn lands as an fp8 hi/lo pair for the
                # DoubleRow out-projection: hi = fp8(a), lo = fp8(a - hi).
                # The multiply runs in place on au (bf16) to save SBUF.
                with nc.allow_low_precision(
                        reason="fp8 hi/lo split, compensated"):
                    nc.vector.tensor_tensor(au[:], au[:], recb[:], mult)
                    nc.scalar.copy(attnT[:, h, qsl], au[:])
                    nc.vector.tensor_tensor(attnT[:, HPC + h, qsl], au[:],
                                            attnT[:, h, qsl],
                                            mybir.AluOpType.subtract)

            def outproj_blocks(pairs, depth=2, evac_dve=False,
                               split_last=False, alt_dma=False):
                # flat generator of out-proj (oc, tt) blocks over the given
                # (jc, oc) pairs; drained one block per chain step so PE
                # never idles while exp paces the chains. wo weight tiles
                # prefetch `depth` oc's ahead (bufs=4 serves both live
                # generators), so no block waits on its weight transfer.
                # PSUM->SBUF evacs go to DVE on the chain-interleaved
                # portion (ACT stays free for the exps the avs wait on) and
                # alternate ACT/DVE on the tail (GPSIMD cannot read PSUM).
                # oc-major across the t-groups: one wo load serves every
                # group's blocks for that column chunk (3x less weight DMA
                # on the interleaved portion)
                def load_wo(oc):
                    wo = pool.tile([128, 2 * ICH, 512], F8, tag="wo", bufs=4)
                    osl = slice(oc * 512, (oc + 1) * 512)
                    nc.sync.dma_start(
                        wo[:, 0:ICH, :],
                        wout8_d[0:768, osl].rearrange("(x p) o -> p x o",
                                                      p=128))
                    nc.sync.dma_start(
                        wo[:, ICH:2 * ICH, :],
                        wout8_d[768:1536, osl].rearrange("(x p) o -> p x o",
                                                         p=128))
                    return wo

                # distinct oc sequence for prefetch (depth 2, bufs=3)
                oc_seq = []
                for _, oc in pairs:
                    if not oc_seq or oc_seq[-1] != oc:
                        oc_seq.append(oc)
                pending = [load_wo(oc) for oc in oc_seq[:depth]]
                nfetched = [min(depth, len(oc_seq))]

                def block(out_ps, wo, t):
                    tsl8 = slice(t * 128, (t + 1) * 128)
                    for i2 in range(ICH // 2):
                        i = 2 * i2
                        ah = attnT[:, i:i + 2, tsl8]
                        al = attnT[:, HPC + i:HPC + i + 2, tsl8]
                        wh_ = wo[:, i:i + 2, :]
                        wl_ = wo[:, ICH + i:ICH + i + 2, :]
                        nc.tensor.matmul(out_ps[:], ah, wh_,
                                         start=(i2 == 0), stop=False,
                                         perf_mode=DRM)
                        nc.tensor.matmul(out_ps[:], ah, wl_,
                                         start=False, stop=False,
                                         perf_mode=DRM)
                        nc.tensor.matmul(out_ps[:], al, wh_,
                                         start=False,
                                         stop=(i2 == ICH // 2 - 1),
                                         perf_mode=DRM)

                def gen():
                    wo = None
                    last_oc = None
                    for n, (jc, oc) in enumerate(pairs):
                        if oc != last_oc:
                            if nfetched[0] < len(oc_seq):
                                pending.append(load_wo(oc_seq[nfetched[0]]))
                                nfetched[0] += 1
                            wo = pending.pop(0)
                            last_oc = oc
                        osl = slice(oc * 512, (oc + 1) * 512)
                        for tt in range(4):
                            t = 4 * jc + tt
                            out_ps = psum.tile([128, 512], F32,
                                               tag="bank", bufs=2)
                            block(out_ps, wo, t)
                            osb = pool.tile([128, 512], F16, tag="osb",
                                            bufs=4)
                            th = slice(jc * 512 + tt * 128,
                                       jc * 512 + (tt + 1) * 128)
                            if (evac_dve and jc < TCH - 1) or \
                                    (oc + tt) % 2 != 0:
                                nc.vector.tensor_copy(osb[:], out_ps[:])
                            else:
                                nc.scalar.copy(osb[:], out_ps[:])
                            if (split_last and n == len(pairs) - 1
                                    and tt == 3):
                                # the kernel's very last transfer: two
                                # halves on separate DGE queues so
                                # gen+transfer run in parallel
                                mid = oc * 512 + 256
                                nc.sync.dma_start(
                                    outp_d[th, oc * 512:mid], osb[:, 0:256])
                                nc.scalar.dma_start(
                                    outp_d[th, mid:(oc + 1) * 512],
                                    osb[:, 256:512])
                            elif alt_dma and (oc + tt) % 2 == 0:
                                # tail: split the output stream across the
                                # SP and ACT DGE queues — one queue alone
                                # backlogs ~1.7us past the last issue
                                nc.scalar.dma_start(outp_d[th, osl], osb[:])
                            else:
                                nc.sync.dma_start(outp_d[th, osl], osb[:])
                            yield

                return gen()

            # ---- Sweeps with the previous chunk's chains interleaved
            # into pass B; post-QKV: last chunk's chains with all out-proj
            # groups interleaved into the PE stream. Chains run 2-way (one
            # head per attnw half): each head's exp gets the other head's
            # scores/av as extra PE filler, halving exp-latency stalls ----
            def chain_group(jc):
                for h0 in range(0, HPC, 2):
                    attnw = psum.tile([128, 1024], F32, tag="attnw", bufs=1)
                    gens = [attn_chain(h0, jc, attnw[:, 0:512], "a"),
                            attn_chain(h0 + 1, jc, attnw[:, 512:1024], "b")]
                    alive = [True, True]
                    while alive[0] or alive[1]:
                        for i, g in enumerate(gens):
                            if alive[i]:
                                try:
                                    next(g)
                                    yield
                                except StopIteration:
                                    alive[i] = False

            load_tables()
            qkv_sweep(0)
            for tcx in range(1, TCH):
                qkv_sweep(tcx, interleave=chain_group(tcx - 1))
            op_pairs = [(jc, oc) for oc in range(OCH)
                        for jc in range(TCH - 1)]
            ops = outproj_blocks(op_pairs, evac_dve=True)
            due = 0.0
            rate = 0.7 * (3 * 4 * OCH) / (HPC * ((4 * 3 + 4) // 2 + 1))
            for _ in chain_group(TCH - 1):
                due += rate
                while due >= 1.0:
                    next(ops, None)
                    due -= 1.0
            # create the last group's generator before draining the rest so
            # its first weight tile is already in flight
            tail = outproj_blocks([(TCH - 1, oc) for oc in range(OCH)],
                                  depth=2, split_last=True, alt_dma=True)
            for _ in ops:
                pass
            for _ in tail:
                pass

    nc.compile()
    return nc


def kernel(hidden_states, position_ids, Wqkv, Wout):
    global _compiled
    hidden_states = np.asarray(hidden_states, dtype=np.float32)
    position_ids = np.asarray(position_ids).astype(np.int64)
    Wqkv = np.asarray(Wqkv, dtype=np.float32)
    Wout = np.asarray(Wout, dtype=np.float32)

    if _compiled is None:
        _compiled = _build()
    nc = _compiled

    import ml_dtypes
    E4M3 = ml_dtypes.float8_e4m3

    def split8(x):
        hi = x.astype(E4M3)
        lo = (x - hi.astype(np.float32)).astype(E4M3)
        return np.ascontiguousarray(hi), np.ascontiguousarray(lo)

    # host prep: rope tables (from actual position_ids), masks, shards.
    # Wqkv is pre-scaled by A_SCALE for the fp8 split; the q tables fold
    # score_scale/A_SCALE, the k tables 1/A_SCALE, and the v path's factor
    # is divided out of the final host sum.
    scale = HD ** -0.5
    half = HD // 2
    inv_freq = 1.0 / (THETA ** (np.arange(half, dtype=np.float64) / half))
    freqs = position_ids.astype(np.float64)[None, :] * inv_freq[:, None]  # [64, T]
    cos = np.cos(freqs)
    sin = np.sin(freqs)
    cosf = np.concatenate([cos, cos], 0)
    sinf = np.concatenate([-sin, sin], 0)
    cosk = (cosf * (scale / A_SCALE)).astype(np.float16)
    sink = (sinf * (scale / A_SCALE)).astype(np.float16)

    p = np.arange(128)[:, None]
    f = np.arange(128)[None, :]
    masks = (f >= p).astype(np.float16)

    def pack_groups(hi, lo):
        # per-512-row group: [hi rows | lo rows] -> [2*rows, cols]
        ng = hi.shape[0] // 512
        out = np.stack([hi.reshape(ng, 512, -1), lo.reshape(ng, 512, -1)],
                       axis=1)
        return np.ascontiguousarray(out.reshape(2 * hi.shape[0], hi.shape[1]))

    hidT = np.ascontiguousarray(hidden_states.T)
    hid8 = pack_groups(*split8(hidT))

    q_size = N_HEADS * HD
    in_maps = []
    for c in range(N_CORES):
        qrows = Wqkv[c * HPC * HD:(c + 1) * HPC * HD]
        krows = Wqkv[q_size + c * HD:q_size + (c + 1) * HD]
        vrows = Wqkv[q_size + N_KV * HD + c * HD:q_size + N_KV * HD + (c + 1) * HD]
        wqkvT = np.ascontiguousarray(
            np.concatenate([qrows, krows, vrows], 0).T) * A_SCALE
        wq8 = pack_groups(*split8(wqkvT))
        woutT = np.ascontiguousarray(
            Wout[:, c * HPC * HD:(c + 1) * HPC * HD].T) * B_SCALE
        wout8 = np.ascontiguousarray(np.concatenate(split8(woutT), axis=0))
        in_maps.append({
            "hid8": hid8, "wq8": wq8, "wout8": wout8,
            "cosk": cosk, "sink": sink,
            "maskm": masks,
        })

    trace = os.environ.get("DBRX_TRACE", "0") == "1"
    res = run_bass_kernel_spmd(nc, in_maps, core_ids=list(range(N_CORES)),
                               trace=trace)
    kernel.last_result = res

    out = res.results[0]["outp"].astype(np.float32)
    for c in range(1, N_CORES):
        out += res.results[c]["outp"].astype(np.float32)
    # undo the v-path A_SCALE carried through attnT and the Wout B_SCALE
    out /= A_SCALE * B_SCALE
    return out

